# revision 30
# baseline (speedup 1.0000x reference)
"""Trainium2 Bass kernel for nn_AttentionLayer (sparse attention with hop-bias).

Computes, for x:[B,V,D], w_qkv:[D,3D], w_out:[D,D], b_out:[D],
rpe_bias:[H,MAX_HOP+1], hop_matrix:[V,V] (int32 in [0,MAX_HOP]):

    qkv = x @ w_qkv ; q,k,v = split(qkv); heads of size HD = D//H
    scores = q @ k^T ; bias = rpe_bias[:, hop_matrix]
    attn = softmax((scores + bias) * HD**-0.5)
    out = (attn @ v).merge_heads() @ w_out + b_out
    returns (out, attn)

Sharding: head-parallel across the 8 NeuronCores (core h owns head h for all
batches); the final out-projection runs as a second tiny launch sharded by
batch.  All computation on device; the host only shards/relayouts inputs and
re-assembles (transposes/concats) outputs.
"""

import sys, os
from contextlib import ExitStack

sys.path.insert(0, "/opt/trn_rl_repo")

import numpy as np
import ml_dtypes

import concourse.bass as bass
import concourse.bacc as bacc
import concourse.mybir as mybir
import concourse.tile as tile
from concourse import bass_utils
from concourse.alu_op_type import AluOpType
from concourse.masks import make_identity
from concourse import hw_specs as _hw

_orig_gat = _hw.get_activation_tables


def _patched_gat(arch):
    t = dict(_orig_gat(arch))
    for k in ("exp_and_others", "natural_log", "exp_and_friends"):
        if k in t:
            t[k] = set()
    return t


bacc.get_activation_tables = _patched_gat

BF16 = ml_dtypes.bfloat16

B, V, D = 8, 1024, 512
H = 8
HD = D // H  # 64
MAX_HOP = 8
NHOP = MAX_HOP + 1  # 9
SCALE = HD ** -0.5  # 0.125
NCORES = 8
JC = V // 128  # 8 j-chunks
F32 = mybir.dt.float32
BF = mybir.dt.bfloat16
GPSIMD_NORM_JC = int(os.environ.get("GPSIMD_NORM_JC", "0"))

_cache = {}


def _build_main():
    """Main SPMD kernel: per core = one head, all batches.

    DRAM inputs (per core):
      xt   [B, 4, 128, V]  bf16   x^T per batch, D split into 4 chunks of 128
      hopt [JC, 128, V]    bf16   hop_matrix^T (j on rows), j split in chunks
      p9   [128, 16]       f32    rpe_bias[h, c] broadcast, c<9
      wqk  [4, 128, 128]   bf16   [w_q_h | w_k_h] as lhsT chunks (d, col)
      wv   [4, 128, HD]    bf16   w_v_h chunks
    DRAM outputs:
      attnT [B, JC, 128, V] bf16  attn^T: [b, j, i] (host transposes+casts)
      oht   [B, HD, V]      bf16  (attn @ v)^T per batch = [d, i]
    """
    nc = bacc.Bacc("TRN2", target_bir_lowering=False, debug=False)

    xt = nc.dram_tensor("xt", [B, 4, 128, V], BF, kind="ExternalInput")
    hopt = nc.dram_tensor("hopt", [JC, 128, V], BF, kind="ExternalInput")
    p9 = nc.dram_tensor("p9", [128, 16], F32, kind="ExternalInput")
    wqk = nc.dram_tensor("wqk", [4, 128, 128], BF, kind="ExternalInput")
    wv = nc.dram_tensor("wv", [4, 128, HD], BF, kind="ExternalInput")
    attnT = nc.dram_tensor("attnT", [B, JC, 128, V], BF, kind="ExternalOutput")
    oht = nc.dram_tensor("oht", [B, HD, V], BF, kind="ExternalOutput")
    rs_scr = nc.dram_tensor("rs_scr", [B, 1, V], F32)

    Exp = mybir.ActivationFunctionType.Exp
    Ln = mybir.ActivationFunctionType.Ln

    with tile.TileContext(nc) as tc, ExitStack() as ctx:
        const = ctx.enter_context(tc.tile_pool(name="const", bufs=1))
        gpool = ctx.enter_context(tc.tile_pool(name="gather", bufs=2))
        xpool = ctx.enter_context(tc.tile_pool(name="x", bufs=2))
        qkp = ctx.enter_context(tc.tile_pool(name="qk", bufs=2))
        ewtp = ctx.enter_context(tc.tile_pool(name="ewt", bufs=3))
        aop = ctx.enter_context(tc.tile_pool(name="ao", bufs=4))
        rsp = ctx.enter_context(tc.tile_pool(name="rs", bufs=2))
        ohp = ctx.enter_context(tc.tile_pool(name="oh", bufs=2))
        psST = ctx.enter_context(tc.tile_pool(name="psST", bufs=5, space="PSUM"))
        psPROJ = ctx.enter_context(tc.tile_pool(name="psPROJ", bufs=1, space="PSUM"))
        psPV = ctx.enter_context(tc.tile_pool(name="psPV", bufs=1, space="PSUM"))

        # ---- constants ----
        p9_sb = const.tile([128, 16], F32)
        nc.sync.dma_start(out=p9_sb, in_=p9[:, :])
        ident128 = const.tile([128, 128], BF)
        make_identity(nc, ident128)

        # ---- gather: LT[j, i] = rpe[h, hop[i, j]] (bf16), added pre-exp ----
        hop_sb = const.tile([128, JC, V], BF)
        nc.sync.dma_start(
            out=hop_sb, in_=hopt[:, :, :].rearrange("c p n -> p c n")
        )
        pt = const.tile([128, JC, V], BF)
        pt2 = pt.rearrange("p c n -> p (c n)")
        hop2 = hop_sb.rearrange("p c n -> p (c n)")
        GCH = 4
        gw = JC * V // GCH
        for g in range(GCH):
            gsl = slice(g * gw, (g + 1) * gw)
            adder = nc.vector
            for c in range(NHOP):
                if c == 0:
                    nc.vector.tensor_scalar(
                        out=pt2[:, gsl], in0=hop2[:, gsl], scalar1=float(c),
                        scalar2=p9_sb[:, c : c + 1],
                        op0=AluOpType.is_equal, op1=AluOpType.mult,
                    )
                else:
                    t = gpool.tile([128, gw], BF, tag="gt")
                    nc.vector.tensor_scalar(
                        out=t, in0=hop2[:, gsl], scalar1=float(c),
                        scalar2=p9_sb[:, c : c + 1],
                        op0=AluOpType.is_equal, op1=AluOpType.mult,
                    )
                    adder.tensor_add(pt2[:, gsl], pt2[:, gsl], t)

        # ---- weights ----
        wqk_sb = const.tile([128, 4, 128], BF)
        nc.sync.dma_start(out=wqk_sb, in_=wqk[:, :, :].rearrange("c p n -> p c n"))
        wv_sb = const.tile([128, 4, HD], BF)
        nc.sync.dma_start(out=wv_sb, in_=wv[:, :, :].rearrange("c p n -> p c n"))

        # ---------- software-pipelined batch loop ----------
        # per-batch state carried between stages
        st_state = {}

        def emit_proj(b):
            xt_sb = xpool.tile([128, 4, V], BF, tag="xt")
            nc.sync.dma_start(
                out=xt_sb, in_=xt[b, :, :, :].rearrange("c p n -> p c n")
            )
            qT = qkp.tile([64, V], BF, tag="qT")
            kT = qkp.tile([64, V], BF, tag="kT")
            for ih in range(2):
                sl = slice(ih * 512, (ih + 1) * 512)
                ps_qk = psPROJ.tile([128, 512], F32, tag="proj")
                for kc in range(4):
                    nc.tensor.matmul(
                        ps_qk,
                        lhsT=wqk_sb[:, kc, :],
                        rhs=xt_sb[:, kc, sl],
                        start=(kc == 0), stop=(kc == 3),
                    )
                nc.scalar.copy(out=qT[:, sl], in_=ps_qk[0:64, :])
                nc.scalar.copy(out=kT[:, sl], in_=ps_qk[64:128, :])

            ps_v = psPROJ.tile([128, 512], F32, tag="proj")
            for t8 in range(8):
                for kc in range(4):
                    nc.tensor.matmul(
                        ps_v[:, t8 * HD : (t8 + 1) * HD],
                        lhsT=xt_sb[:, kc, t8 * 128 : (t8 + 1) * 128],
                        rhs=wv_sb[:, kc, :],
                        start=(t8 == 0 and kc == 0), stop=(t8 == 7 and kc == 3),
                    )
            v65 = qkp.tile([128, JC, HD + 1], BF, tag="v65")
            nc.vector.memset(v65[:, :, HD : HD + 1], 1.0)
            nc.scalar.copy(
                out=v65[:, :, 0:HD], in_=ps_v.rearrange("p (c d) -> p c d", d=HD)
            )
            st_state[b] = dict(qT=qT, kT=kT, v65=v65)

        def emit_attention(b):
            s = st_state[b]
            qT, kT, v65 = s["qT"], s["kT"], s["v65"]
            ewt = ewtp.tile([128, JC, V], BF, tag="ewt")
            ps_pv = psPV.tile([65, V], F32, tag="pv")
            s["ewt"], s["ps_pv"] = ewt, ps_pv
            LAG = 2

            def emit_pv(jc):
                for ih in range(2):
                    sl = slice(ih * 512, (ih + 1) * 512)
                    nc.tensor.matmul(
                        ps_pv[:, sl],
                        lhsT=v65[:, jc, :],
                        rhs=ewt[:, jc, sl],
                        start=(jc == 0), stop=(jc == JC - 1),
                    )

            for jg in range(0, JC, 2):
                tiles = []
                for j2 in (jg, jg + 1):
                    for ih in range(2):
                        sl = slice(ih * 512, (ih + 1) * 512)
                        ps_st = psST.tile([128, 512], F32, tag="st")
                        nc.tensor.matmul(
                            ps_st, lhsT=ident128, rhs=pt[:, j2, sl],
                            start=True, stop=False,
                        )
                        tiles.append((j2, sl, ps_st))
                for j2, sl, ps_st in tiles:
                    nc.tensor.matmul(
                        ps_st,
                        lhsT=kT[:, j2 * 128 : (j2 + 1) * 128],
                        rhs=qT[:, sl],
                        start=False, stop=True,
                    )
                for j2, sl, ps_st in tiles:
                    nc.scalar.activation(
                        out=ewt[:, j2, sl], in_=ps_st, func=Exp, scale=SCALE
                    )
                if jg >= 2:
                    emit_pv(jg - 2)
                    emit_pv(jg - 1)
            for jc in range(JC - 2, JC):
                emit_pv(jc)

        def emit_tail(b):
            s = st_state[b]
            ewt, ps_pv = s["ewt"], s["ps_pv"]
            # rowsum reciprocal via exp(-ln)
            lnrs = rsp.tile([1, V], F32, tag="lnrs")
            nc.scalar.activation(out=lnrs, in_=ps_pv[64:65, :], func=Ln)
            rs_inv = rsp.tile([1, V], F32, tag="rsinv")
            nc.scalar.activation(out=rs_inv, in_=lnrs, func=Exp, scale=-1.0)
            # broadcast via DRAM bounce
            nc.sync.dma_start(out=rs_scr[b, :, :], in_=rs_inv)
            rs_rep = rsp.tile([128, V], BF, tag="rsrep")
            rs_bcast = bass.AP(
                tensor=rs_scr.ap().tensor,
                offset=rs_scr.ap().offset + b * V,
                ap=[[0, 128], [1, V]],
            )
            nc.gpsimd.dma_start(out=rs_rep, in_=rs_bcast)
            # oht (normalized by 1/rowsum along free dim)
            oht_sb = ohp.tile([HD, V], BF, tag="oht")
            nc.vector.tensor_mul(oht_sb, ps_pv[0:HD, :], rs_rep[0:HD, :])
            nc.sync.dma_start(out=oht[b, :, :], in_=oht_sb)
            # normalize + write attn^T
            for jc in range(JC):
                ao = aop.tile([128, V], BF, tag="ao")
                if jc >= JC - GPSIMD_NORM_JC:
                    nc.gpsimd.tensor_mul(ao, ewt[:, jc, :], rs_rep)
                else:
                    nc.vector.tensor_mul(ao, ewt[:, jc, :], rs_rep)
                nc.sync.dma_start(out=attnT[b, jc, :, :], in_=ao)
            del st_state[b]

        for b in range(B + 1):
            if b < B:
                emit_proj(b)
            if b >= 1:
                emit_tail(b - 1)
            if b < B:
                emit_attention(b)

    nc.compile()
    return nc


def _build_proj():
    """Out-projection kernel: per core = one batch.

    Inputs: ohtb [4, 128, V] bf16 (OHT_b, d-chunks), wout [4, 128, D] bf16,
            bout [128, D] f32 (pre-broadcast).
    Output: o [8, 128, D] f32 (tok-chunks of out[b]).
    """
    nc = bacc.Bacc("TRN2", target_bir_lowering=False, debug=False)
    ohtb = nc.dram_tensor("ohtb", [4, 128, V], BF, kind="ExternalInput")
    wout = nc.dram_tensor("wout", [4, 128, D], BF, kind="ExternalInput")
    bout = nc.dram_tensor("bout", [128, D], F32, kind="ExternalInput")
    o = nc.dram_tensor("o", [8, 128, D], F32, kind="ExternalOutput")

    with tile.TileContext(nc) as tc, ExitStack() as ctx:
        const = ctx.enter_context(tc.tile_pool(name="const", bufs=1))
        op = ctx.enter_context(tc.tile_pool(name="o", bufs=3))
        psp = ctx.enter_context(tc.tile_pool(name="ps", bufs=2, space="PSUM"))

        oht_sb = const.tile([128, 4, V], BF)
        nc.sync.dma_start(out=oht_sb, in_=ohtb[:, :, :].rearrange("c p n -> p c n"))
        wout_sb = const.tile([128, 4, D], BF)
        nc.sync.dma_start(out=wout_sb, in_=wout[:, :, :].rearrange("c p n -> p c n"))
        bout_sb = const.tile([128, D], F32)
        nc.sync.dma_start(out=bout_sb, in_=bout[:, :])

        for t8 in range(8):
            ps = psp.tile([128, D], F32, tag="ps")
            for kc in range(4):
                nc.tensor.matmul(
                    ps,
                    lhsT=oht_sb[:, kc, t8 * 128 : (t8 + 1) * 128],
                    rhs=wout_sb[:, kc, :],
                    start=(kc == 0), stop=(kc == 3),
                )
            o_sb = op.tile([128, D], F32, tag="osb")
            nc.vector.tensor_add(o_sb, ps, bout_sb)
            nc.sync.dma_start(out=o[t8, :, :], in_=o_sb)

    nc.compile()
    return nc


def _get_kernels():
    if "main" not in _cache:
        _cache["main"] = _build_main()
        _cache["proj"] = _build_proj()
    return _cache["main"], _cache["proj"]


def kernel(x, w_qkv, w_out, b_out, rpe_bias, hop_matrix):
    x = np.asarray(x, dtype=np.float32)
    w_qkv = np.asarray(w_qkv, dtype=np.float32)
    w_out = np.asarray(w_out, dtype=np.float32)
    b_out = np.asarray(b_out, dtype=np.float32)
    rpe_bias = np.asarray(rpe_bias, dtype=np.float32)
    hop_matrix = np.asarray(hop_matrix)
    hop_dtype = hop_matrix.dtype

    nc_main, nc_proj = _get_kernels()

    # ---- host-side sharding / relayout ----
    xt = np.ascontiguousarray(x.transpose(0, 2, 1)).reshape(B, 4, 128, V)
    xt = xt.astype(BF16)
    hopt = (
        np.ascontiguousarray(hop_matrix.T.astype(np.float32))
        .reshape(JC, 128, V)
        .astype(BF16)
    )
    wq = w_qkv[:, 0:D]
    wk = w_qkv[:, D : 2 * D]
    wv_full = w_qkv[:, 2 * D : 3 * D]

    in_maps = []
    for h in range(NCORES):
        p9v = rpe_bias[h, :].astype(np.float32)  # raw bias values (added pre-exp)
        p9a = np.zeros((128, 16), np.float32)
        p9a[:, :NHOP] = p9v[None, :]
        wqk_h = np.concatenate(
            [wq[:, h * HD : (h + 1) * HD], wk[:, h * HD : (h + 1) * HD]], axis=1
        )  # [512, 128]
        wqk_h = np.ascontiguousarray(wqk_h).reshape(4, 128, 128).astype(BF16)
        wv_h = (
            np.ascontiguousarray(wv_full[:, h * HD : (h + 1) * HD])
            .reshape(4, 128, HD)
            .astype(BF16)
        )
        in_maps.append(
            {"xt": xt, "hopt": hopt, "p9": p9a, "wqk": wqk_h, "wv": wv_h}
        )

    res1 = bass_utils.run_bass_kernel_spmd(
        nc_main, in_maps, core_ids=list(range(NCORES))
    )

    # ---- assemble attn ----
    attn = np.empty((B, H, V, V), np.float32)
    for h in range(NCORES):
        at = np.asarray(res1.results[h]["attnT"]).astype(np.float32).reshape(B, V, V)
        attn[:, h] = at.transpose(0, 2, 1)

    # ---- second launch: out projection (batch-sharded) ----
    wout_in = np.ascontiguousarray(w_out).reshape(4, 128, D).astype(BF16)
    bout_in = np.broadcast_to(b_out, (128, D)).astype(np.float32).copy()
    in_maps2 = []
    for b in range(B):
        ohtb = np.concatenate(
            [np.asarray(res1.results[h]["oht"][b]) for h in range(H)], axis=0
        )  # [512, V] bf16
        ohtb = np.ascontiguousarray(ohtb).reshape(4, 128, V)
        in_maps2.append({"ohtb": ohtb, "wout": wout_in, "bout": bout_in})

    res2 = bass_utils.run_bass_kernel_spmd(
        nc_proj, in_maps2, core_ids=list(range(NCORES))
    )
    out = np.stack(
        [np.asarray(res2.results[b]["o"], np.float32).reshape(V, D) for b in range(B)]
    )
    return out, attn


# revision 31
# speedup vs baseline: 1.0398x; 1.0398x over previous
"""Trainium2 Bass kernel for nn_AttentionLayer (sparse attention with hop-bias).

Computes, for x:[B,V,D], w_qkv:[D,3D], w_out:[D,D], b_out:[D],
rpe_bias:[H,MAX_HOP+1], hop_matrix:[V,V] (int32 in [0,MAX_HOP]):

    qkv = x @ w_qkv ; q,k,v = split(qkv); heads of size HD = D//H
    scores = q @ k^T ; bias = rpe_bias[:, hop_matrix]
    attn = softmax((scores + bias) * HD**-0.5)
    out = (attn @ v).merge_heads() @ w_out + b_out
    returns (out, attn)

Sharding: head-parallel across the 8 NeuronCores (core h owns head h for all
batches); the final out-projection runs as a second tiny launch sharded by
batch.  All computation on device; the host only shards/relayouts inputs and
re-assembles (transposes/concats) outputs.
"""

import sys, os
from contextlib import ExitStack

sys.path.insert(0, "/opt/trn_rl_repo")

import numpy as np
import ml_dtypes

import concourse.bass as bass
import concourse.bacc as bacc
import concourse.mybir as mybir
import concourse.tile as tile
from concourse import bass_utils
from concourse.alu_op_type import AluOpType
from concourse.masks import make_identity
from concourse import hw_specs as _hw

_orig_gat = _hw.get_activation_tables


def _patched_gat(arch):
    t = dict(_orig_gat(arch))
    for k in ("exp_and_others", "natural_log", "exp_and_friends"):
        if k in t:
            t[k] = set()
    return t


bacc.get_activation_tables = _patched_gat

BF16 = ml_dtypes.bfloat16

B, V, D = 8, 1024, 512
H = 8
HD = D // H  # 64
MAX_HOP = 8
NHOP = MAX_HOP + 1  # 9
SCALE = HD ** -0.5  # 0.125
NCORES = 8
JC = V // 128  # 8 j-chunks
F32 = mybir.dt.float32
BF = mybir.dt.bfloat16
GPSIMD_NORM_JC = int(os.environ.get("GPSIMD_NORM_JC", "0"))

_cache = {}


def _build_main():
    """Main SPMD kernel: per core = one head, all batches.

    DRAM inputs (per core):
      xt   [B, 4, 128, V]  bf16   x^T per batch, D split into 4 chunks of 128
      hopt [JC, 128, V]    bf16   hop_matrix^T (j on rows), j split in chunks
      p9   [128, 16]       f32    rpe_bias[h, c] broadcast, c<9
      wqk  [4, 128, 128]   bf16   [w_q_h | w_k_h] as lhsT chunks (d, col)
      wv   [4, 128, HD]    bf16   w_v_h chunks
    DRAM outputs:
      attnT [B, JC, 128, V] bf16  attn^T: [b, j, i] (host transposes+casts)
      oht   [B, HD, V]      bf16  (attn @ v)^T per batch = [d, i]
    """
    nc = bacc.Bacc("TRN2", target_bir_lowering=False, debug=False)

    xt = nc.dram_tensor("xt", [B, 4, 128, V], BF, kind="ExternalInput")
    hopt = nc.dram_tensor("hopt", [JC, 128, V], BF, kind="ExternalInput")
    p9 = nc.dram_tensor("p9", [128, 16], F32, kind="ExternalInput")
    wqk = nc.dram_tensor("wqk", [4, 128, 128], BF, kind="ExternalInput")
    wv = nc.dram_tensor("wv", [4, 128, HD], BF, kind="ExternalInput")
    attnT = nc.dram_tensor("attnT", [B, JC, 128, V], BF, kind="ExternalOutput")
    oht = nc.dram_tensor("oht", [B, HD, V], BF, kind="ExternalOutput")
    rs_scr = nc.dram_tensor("rs_scr", [B, 1, V], F32)

    Exp = mybir.ActivationFunctionType.Exp
    Ln = mybir.ActivationFunctionType.Ln

    with tile.TileContext(nc) as tc, ExitStack() as ctx:
        const = ctx.enter_context(tc.tile_pool(name="const", bufs=1))
        gpool = ctx.enter_context(tc.tile_pool(name="gather", bufs=2))
        xpool = ctx.enter_context(tc.tile_pool(name="x", bufs=3))
        qkp = ctx.enter_context(tc.tile_pool(name="qk", bufs=2))
        ewtp = ctx.enter_context(tc.tile_pool(name="ewt", bufs=3))
        aop = ctx.enter_context(tc.tile_pool(name="ao", bufs=4))
        rsp = ctx.enter_context(tc.tile_pool(name="rs", bufs=2))
        ohp = ctx.enter_context(tc.tile_pool(name="oh", bufs=2))
        psST = ctx.enter_context(tc.tile_pool(name="psST", bufs=5, space="PSUM"))
        psPROJ = ctx.enter_context(tc.tile_pool(name="psPROJ", bufs=1, space="PSUM"))
        psPV = ctx.enter_context(tc.tile_pool(name="psPV", bufs=1, space="PSUM"))

        # ---- constants ----
        p9_sb = const.tile([128, 16], F32)
        nc.sync.dma_start(out=p9_sb, in_=p9[:, :])
        ident128 = const.tile([128, 128], BF)
        make_identity(nc, ident128)

        # ---- gather: LT[j, i] = rpe[h, hop[i, j]] (bf16), added pre-exp ----
        hop_sb = const.tile([128, JC, V], BF)
        nc.sync.dma_start(
            out=hop_sb, in_=hopt[:, :, :].rearrange("c p n -> p c n")
        )
        pt = const.tile([128, JC, V], BF)
        pt2 = pt.rearrange("p c n -> p (c n)")
        hop2 = hop_sb.rearrange("p c n -> p (c n)")
        GCH = 4
        gw = JC * V // GCH
        for g in range(GCH):
            gsl = slice(g * gw, (g + 1) * gw)
            adder = nc.vector
            for c in range(NHOP):
                if c == 0:
                    nc.vector.tensor_scalar(
                        out=pt2[:, gsl], in0=hop2[:, gsl], scalar1=float(c),
                        scalar2=p9_sb[:, c : c + 1],
                        op0=AluOpType.is_equal, op1=AluOpType.mult,
                    )
                else:
                    t = gpool.tile([128, gw], BF, tag="gt")
                    nc.vector.tensor_scalar(
                        out=t, in0=hop2[:, gsl], scalar1=float(c),
                        scalar2=p9_sb[:, c : c + 1],
                        op0=AluOpType.is_equal, op1=AluOpType.mult,
                    )
                    adder.tensor_add(pt2[:, gsl], pt2[:, gsl], t)

        # ---- weights ----
        wqk_sb = const.tile([128, 4, 128], BF)
        nc.sync.dma_start(out=wqk_sb, in_=wqk[:, :, :].rearrange("c p n -> p c n"))
        wv_sb = const.tile([128, 4, HD], BF)
        nc.sync.dma_start(out=wv_sb, in_=wv[:, :, :].rearrange("c p n -> p c n"))

        # ---------- software-pipelined batch loop ----------
        # per-batch state carried between stages
        st_state = {}

        def emit_proj(b):
            xt_sb = xpool.tile([128, 4, V], BF, tag="xt")
            nc.sync.dma_start(
                out=xt_sb, in_=xt[b, :, :, :].rearrange("c p n -> p c n")
            )
            qT = qkp.tile([64, V], BF, tag="qT")
            kT = qkp.tile([64, V], BF, tag="kT")
            for ih in range(2):
                sl = slice(ih * 512, (ih + 1) * 512)
                ps_qk = psPROJ.tile([128, 512], F32, tag="proj")
                for kc in range(4):
                    nc.tensor.matmul(
                        ps_qk,
                        lhsT=wqk_sb[:, kc, :],
                        rhs=xt_sb[:, kc, sl],
                        start=(kc == 0), stop=(kc == 3),
                    )
                nc.scalar.copy(out=qT[:, sl], in_=ps_qk[0:64, :])
                nc.scalar.copy(out=kT[:, sl], in_=ps_qk[64:128, :])

            ps_v = psPROJ.tile([128, 512], F32, tag="proj")
            for t8 in range(8):
                for kc in range(4):
                    nc.tensor.matmul(
                        ps_v[:, t8 * HD : (t8 + 1) * HD],
                        lhsT=xt_sb[:, kc, t8 * 128 : (t8 + 1) * 128],
                        rhs=wv_sb[:, kc, :],
                        start=(t8 == 0 and kc == 0), stop=(t8 == 7 and kc == 3),
                    )
            v65 = qkp.tile([128, JC, HD + 1], BF, tag="v65")
            nc.vector.memset(v65[:, :, HD : HD + 1], 1.0)
            nc.scalar.copy(
                out=v65[:, :, 0:HD], in_=ps_v.rearrange("p (c d) -> p c d", d=HD)
            )
            st_state[b] = dict(qT=qT, kT=kT, v65=v65)

        def emit_attention(b):
            s = st_state[b]
            qT, kT, v65 = s["qT"], s["kT"], s["v65"]
            ewt = ewtp.tile([128, JC, V], BF, tag="ewt")
            ps_pv = psPV.tile([65, V], F32, tag="pv")
            s["ewt"], s["ps_pv"] = ewt, ps_pv
            LAG = 2

            def emit_pv(jc):
                for ih in range(2):
                    sl = slice(ih * 512, (ih + 1) * 512)
                    nc.tensor.matmul(
                        ps_pv[:, sl],
                        lhsT=v65[:, jc, :],
                        rhs=ewt[:, jc, sl],
                        start=(jc == 0), stop=(jc == JC - 1),
                    )

            for jg in range(0, JC, 2):
                tiles = []
                for j2 in (jg, jg + 1):
                    for ih in range(2):
                        sl = slice(ih * 512, (ih + 1) * 512)
                        ps_st = psST.tile([128, 512], F32, tag="st")
                        nc.tensor.matmul(
                            ps_st, lhsT=ident128, rhs=pt[:, j2, sl],
                            start=True, stop=False,
                        )
                        tiles.append((j2, sl, ps_st))
                for j2, sl, ps_st in tiles:
                    nc.tensor.matmul(
                        ps_st,
                        lhsT=kT[:, j2 * 128 : (j2 + 1) * 128],
                        rhs=qT[:, sl],
                        start=False, stop=True,
                    )
                for j2, sl, ps_st in tiles:
                    nc.scalar.activation(
                        out=ewt[:, j2, sl], in_=ps_st, func=Exp, scale=SCALE
                    )
                if jg >= 2:
                    emit_pv(jg - 2)
                    emit_pv(jg - 1)
            for jc in range(JC - 2, JC):
                emit_pv(jc)

        def emit_tail(b):
            s = st_state[b]
            ewt, ps_pv = s["ewt"], s["ps_pv"]
            # rowsum reciprocal via exp(-ln)
            lnrs = rsp.tile([1, V], F32, tag="lnrs")
            nc.scalar.activation(out=lnrs, in_=ps_pv[64:65, :], func=Ln)
            rs_inv = rsp.tile([1, V], F32, tag="rsinv")
            nc.scalar.activation(out=rs_inv, in_=lnrs, func=Exp, scale=-1.0)
            # broadcast via DRAM bounce
            nc.sync.dma_start(out=rs_scr[b, :, :], in_=rs_inv)
            rs_rep = rsp.tile([128, V], BF, tag="rsrep")
            rs_bcast = bass.AP(
                tensor=rs_scr.ap().tensor,
                offset=rs_scr.ap().offset + b * V,
                ap=[[0, 128], [1, V]],
            )
            nc.gpsimd.dma_start(out=rs_rep, in_=rs_bcast)
            # oht (normalized by 1/rowsum along free dim)
            oht_sb = ohp.tile([HD, V], BF, tag="oht")
            nc.vector.tensor_mul(oht_sb, ps_pv[0:HD, :], rs_rep[0:HD, :])
            nc.sync.dma_start(out=oht[b, :, :], in_=oht_sb)
            # normalize + write attn^T
            for jc in range(JC):
                ao = aop.tile([128, V], BF, tag="ao")
                if jc >= JC - GPSIMD_NORM_JC:
                    nc.gpsimd.tensor_mul(ao, ewt[:, jc, :], rs_rep)
                else:
                    nc.vector.tensor_mul(ao, ewt[:, jc, :], rs_rep)
                nc.sync.dma_start(out=attnT[b, jc, :, :], in_=ao)
            del st_state[b]

        for b in range(B + 1):
            if b < B:
                emit_proj(b)
            if b >= 1:
                emit_tail(b - 1)
            if b < B:
                emit_attention(b)

    nc.compile()
    return nc


def _build_proj():
    """Out-projection kernel: per core = one batch.

    Inputs: ohtb [4, 128, V] bf16 (OHT_b, d-chunks), wout [4, 128, D] bf16,
            bout [128, D] f32 (pre-broadcast).
    Output: o [8, 128, D] f32 (tok-chunks of out[b]).
    """
    nc = bacc.Bacc("TRN2", target_bir_lowering=False, debug=False)
    ohtb = nc.dram_tensor("ohtb", [4, 128, V], BF, kind="ExternalInput")
    wout = nc.dram_tensor("wout", [4, 128, D], BF, kind="ExternalInput")
    bout = nc.dram_tensor("bout", [128, D], F32, kind="ExternalInput")
    o = nc.dram_tensor("o", [8, 128, D], F32, kind="ExternalOutput")

    with tile.TileContext(nc) as tc, ExitStack() as ctx:
        const = ctx.enter_context(tc.tile_pool(name="const", bufs=1))
        op = ctx.enter_context(tc.tile_pool(name="o", bufs=3))
        psp = ctx.enter_context(tc.tile_pool(name="ps", bufs=2, space="PSUM"))

        wout_sb = const.tile([128, 4, D], BF)
        nc.sync.dma_start(out=wout_sb, in_=wout[:, :, :].rearrange("c p n -> p c n"))
        bout_sb = const.tile([128, D], F32)
        nc.sync.dma_start(out=bout_sb, in_=bout[:, :])
        oht_sb = const.tile([128, 4, V], BF)
        for th in range(4):
            tsl = slice(th * 256, (th + 1) * 256)
            nc.sync.dma_start(
                out=oht_sb[:, :, tsl],
                in_=ohtb[:, :, tsl].rearrange("c p n -> p c n"),
            )

        for t8 in range(8):
            ps = psp.tile([128, D], F32, tag="ps")
            for kc in range(4):
                nc.tensor.matmul(
                    ps,
                    lhsT=oht_sb[:, kc, t8 * 128 : (t8 + 1) * 128],
                    rhs=wout_sb[:, kc, :],
                    start=(kc == 0), stop=(kc == 3),
                )
            o_sb = op.tile([128, D], F32, tag="osb")
            nc.vector.tensor_add(o_sb, ps, bout_sb)
            nc.sync.dma_start(out=o[t8, :, :], in_=o_sb)

    nc.compile()
    return nc


def _get_kernels():
    if "main" not in _cache:
        _cache["main"] = _build_main()
        _cache["proj"] = _build_proj()
    return _cache["main"], _cache["proj"]


def kernel(x, w_qkv, w_out, b_out, rpe_bias, hop_matrix):
    x = np.asarray(x, dtype=np.float32)
    w_qkv = np.asarray(w_qkv, dtype=np.float32)
    w_out = np.asarray(w_out, dtype=np.float32)
    b_out = np.asarray(b_out, dtype=np.float32)
    rpe_bias = np.asarray(rpe_bias, dtype=np.float32)
    hop_matrix = np.asarray(hop_matrix)
    hop_dtype = hop_matrix.dtype

    nc_main, nc_proj = _get_kernels()

    # ---- host-side sharding / relayout ----
    xt = np.ascontiguousarray(x.transpose(0, 2, 1)).reshape(B, 4, 128, V)
    xt = xt.astype(BF16)
    hopt = (
        np.ascontiguousarray(hop_matrix.T.astype(np.float32))
        .reshape(JC, 128, V)
        .astype(BF16)
    )
    wq = w_qkv[:, 0:D]
    wk = w_qkv[:, D : 2 * D]
    wv_full = w_qkv[:, 2 * D : 3 * D]

    in_maps = []
    for h in range(NCORES):
        p9v = rpe_bias[h, :].astype(np.float32)  # raw bias values (added pre-exp)
        p9a = np.zeros((128, 16), np.float32)
        p9a[:, :NHOP] = p9v[None, :]
        wqk_h = np.concatenate(
            [wq[:, h * HD : (h + 1) * HD], wk[:, h * HD : (h + 1) * HD]], axis=1
        )  # [512, 128]
        wqk_h = np.ascontiguousarray(wqk_h).reshape(4, 128, 128).astype(BF16)
        wv_h = (
            np.ascontiguousarray(wv_full[:, h * HD : (h + 1) * HD])
            .reshape(4, 128, HD)
            .astype(BF16)
        )
        in_maps.append(
            {"xt": xt, "hopt": hopt, "p9": p9a, "wqk": wqk_h, "wv": wv_h}
        )

    res1 = bass_utils.run_bass_kernel_spmd(
        nc_main, in_maps, core_ids=list(range(NCORES))
    )

    # ---- assemble attn ----
    attn = np.empty((B, H, V, V), np.float32)
    for h in range(NCORES):
        at = np.asarray(res1.results[h]["attnT"]).astype(np.float32).reshape(B, V, V)
        attn[:, h] = at.transpose(0, 2, 1)

    # ---- second launch: out projection (batch-sharded) ----
    wout_in = np.ascontiguousarray(w_out).reshape(4, 128, D).astype(BF16)
    bout_in = np.broadcast_to(b_out, (128, D)).astype(np.float32).copy()
    in_maps2 = []
    for b in range(B):
        ohtb = np.concatenate(
            [np.asarray(res1.results[h]["oht"][b]) for h in range(H)], axis=0
        )  # [512, V] bf16
        ohtb = np.ascontiguousarray(ohtb).reshape(4, 128, V)
        in_maps2.append({"ohtb": ohtb, "wout": wout_in, "bout": bout_in})

    res2 = bass_utils.run_bass_kernel_spmd(
        nc_proj, in_maps2, core_ids=list(range(NCORES))
    )
    out = np.stack(
        [np.asarray(res2.results[b]["o"], np.float32).reshape(V, D) for b in range(B)]
    )
    return out, attn
